# revision 4
# baseline (speedup 1.0000x reference)
"""Bass/Trainium2 kernel for the Bigram (1-block transformer) LM problem.

Strategy (8 NeuronCores, SPMD single-program per launch):
  Launch 1: attention + FFN. Core c handles batch b=c//2, row half c%2
            (512 of 1024 rows). All tensors kept transposed ([D, rows]) so
            every matmul consumes natural layouts; causal mask is additive
            data shipped from host. Output: fT slice [1024, 512] f32.
  Launch 2: LM head, vocab-sharded: core c computes
            logits[:, c*VC:(c+1)*VC] = f @ Wlm[:, shard] + blm[shard]
            for all 4096 rows, plus per-row sum(exp(logits_shard)) on-device.
  Host: embeds (gather) + transposes + weight repacking, assembles logits,
        combines per-shard sumexp into the cross-entropy loss.

All matmuls run in float32r (TF32-class, full PE rate at moving-dim >= 256).
"""

import sys
import numpy as np

B, T, D, V = 4, 1024, 1024, 50257
H, DH, FF = 4, 256, 4096
NCORES = 8
VC = -(-V // NCORES)          # 6283 vocab cols per core (last core: 7 pad)
VCD = VC + (VC % 2)           # 6284: device shard width (fp32r needs even N)
ROWS = B * T                  # 4096
RT = ROWS // 128              # 32 row tiles
KT = D // 128                 # 8 contraction tiles over D
MT1 = FF // 128               # 32 FFN-hidden tiles
NEG = -30.0                   # additive mask value (exp(-30) ~ 1e-13)
SCALE = float(D) ** -0.5

_CACHE = {}


def _modules():
    for p in ("/opt/trn_rl_repo", "/root/.axon_site/_ro/trn_rl_repo"):
        if p not in sys.path:
            sys.path.append(p)
    import concourse.bass as bass
    import concourse.mybir as mybir
    from concourse import bacc
    from concourse.tile import TileContext
    from concourse.bass_utils import run_bass_kernel_spmd
    return bass, mybir, bacc, TileContext, run_bass_kernel_spmd


def _build_l1():
    """Attention + FFN for 512 rows of one batch. Inputs are per-core."""
    bass, mybir, bacc, TileContext, _ = _modules()
    f32, f32r = mybir.dt.float32, mybir.dt.float32r
    nc = bacc.Bacc("TRN2", num_devices=NCORES)

    hTq_d = nc.dram_tensor("hTq", [D, 512], f32r, kind="ExternalInput")
    hTk_d = nc.dram_tensor("hTk", [D, T], f32r, kind="ExternalInput")
    wq_d = nc.dram_tensor("wq", [H, D, DH], f32r, kind="ExternalInput")
    wk_d = nc.dram_tensor("wk", [H, D, DH], f32r, kind="ExternalInput")
    wv_d = nc.dram_tensor("wv", [H, D, DH], f32r, kind="ExternalInput")
    mask_d = nc.dram_tensor("maskT", [T, 512], f32, kind="ExternalInput")
    ones_d = nc.dram_tensor("ones", [128, 1], f32r, kind="ExternalInput")
    w1_d = nc.dram_tensor("w1s", [MT1, 128, D], f32r, kind="ExternalInput")
    b1_d = nc.dram_tensor("b1t", [128, MT1], f32, kind="ExternalInput")
    w2_d = nc.dram_tensor("w2s", [KT, 128, FF], f32r, kind="ExternalInput")
    b2_d = nc.dram_tensor("b2t", [128, KT], f32, kind="ExternalInput")
    fT_d = nc.dram_tensor("fT", [D, 512], f32, kind="ExternalOutput")

    Exp = mybir.ActivationFunctionType.Exp
    Relu = mybir.ActivationFunctionType.Relu
    Ident = mybir.ActivationFunctionType.Identity
    Copy = mybir.ActivationFunctionType.Copy

    with TileContext(nc) as tc:
        with tc.tile_pool(name="persist", bufs=1) as pp, \
             tc.tile_pool(name="psmm", bufs=2, space="PSUM") as psmm:

            ones_sb = pp.tile([128, 1], f32r, tag="ones")
            nc.sync.dma_start(ones_sb[:, :], ones_d[:, :])

            with tc.tile_pool(name="qkv", bufs=1) as qkvp:
                qT_sb = qkvp.tile([128, H * 2, 512], f32r, tag="qT")
                kT_sb = qkvp.tile([128, H * 2, T], f32r, tag="kT")
                v_sb = qkvp.tile([128, H * KT, DH], f32r, tag="v")

                # ---- phase A1: qT/kT/v for all heads (hT resident) ----
                with tc.tile_pool(name="hT", bufs=1) as hp, \
                     tc.tile_pool(name="wqkv", bufs=2) as wp:
                    hTq_sb = hp.tile([128, KT, 512], f32r, tag="hTq")
                    nc.sync.dma_start(
                        hTq_sb[:, :, :],
                        hTq_d.ap().rearrange("(a p) n -> p a n", p=128))
                    hTk_sb = hp.tile([128, KT, T], f32r, tag="hTk")
                    nc.sync.dma_start(
                        hTk_sb[:, :, :],
                        hTk_d.ap().rearrange("(a p) n -> p a n", p=128))

                    for h in range(H):
                        wq_sb = wp.tile([128, KT, DH], f32r, tag="wq")
                        nc.sync.dma_start(
                            wq_sb[:, :, :],
                            wq_d[h].rearrange("(a p) e -> p a e", p=128))
                        wk_sb = wp.tile([128, KT, DH], f32r, tag="wk")
                        nc.sync.dma_start(
                            wk_sb[:, :, :],
                            wk_d[h].rearrange("(a p) e -> p a e", p=128))
                        wv_sb = wp.tile([128, KT, DH], f32r, tag="wv")
                        nc.sync.dma_start(
                            wv_sb[:, :, :],
                            wv_d[h].rearrange("(a p) e -> p a e", p=128))

                        # qT[dh-tile m, 512 rows], scaled by D**-0.5
                        for m in range(2):
                            pq = psmm.tile([128, 512], f32, tag="mm")
                            for kt in range(KT):
                                nc.tensor.matmul(
                                    pq[:, :],
                                    wq_sb[:, kt, m * 128:(m + 1) * 128],
                                    hTq_sb[:, kt, :],
                                    start=(kt == 0), stop=(kt == KT - 1))
                            nc.scalar.activation(
                                qT_sb[:, 2 * h + m, :], pq[:, :], Copy,
                                scale=SCALE)
                        # kT over full T (two 512 chunks)
                        for m in range(2):
                            for nch in range(2):
                                pk = psmm.tile([128, 512], f32, tag="mm")
                                for kt in range(KT):
                                    nc.tensor.matmul(
                                        pk[:, :],
                                        wk_sb[:, kt, m * 128:(m + 1) * 128],
                                        hTk_sb[:, kt, nch * 512:(nch + 1) * 512],
                                        start=(kt == 0), stop=(kt == KT - 1))
                                nc.scalar.activation(
                                    kT_sb[:, 2 * h + m,
                                          nch * 512:(nch + 1) * 512],
                                    pk[:, :], Copy)
                        # v [T rows, DH] natural
                        for tt in range(KT):
                            pv = psmm.tile([128, 512], f32, tag="mm")
                            for kt in range(KT):
                                nc.tensor.matmul(
                                    pv[:, 0:DH],
                                    hTk_sb[:, kt, tt * 128:(tt + 1) * 128],
                                    wv_sb[:, kt, :],
                                    start=(kt == 0), stop=(kt == KT - 1))
                            nc.scalar.activation(
                                v_sb[:, h * KT + tt, :], pv[:, 0:DH], Copy)

                # ---- phase A2: scores -> exp -> AV -> normalize ----
                oT_sb = pp.tile([128, KT, 512], f32r, tag="oT")
                with tc.tile_pool(name="maskp", bufs=1) as mp, \
                     tc.tile_pool(name="att", bufs=2) as ap_, \
                     tc.tile_pool(name="psov", bufs=2, space="PSUM") as psov, \
                     tc.tile_pool(name="pss", bufs=2, space="PSUM") as pss:
                    mask_sb = mp.tile([128, KT, 512], f32, tag="mask")
                    nc.sync.dma_start(
                        mask_sb[:, :, :],
                        mask_d.ap().rearrange("(a p) n -> p a n", p=128))

                    for h in range(H):
                        expT_sb = ap_.tile([128, KT, 512], f32r, tag="expT")
                        for kt in range(KT):
                            ps_s = psmm.tile([128, 512], f32, tag="mm")
                            for dh in range(2):
                                nc.tensor.matmul(
                                    ps_s[:, :],
                                    kT_sb[:, 2 * h + dh,
                                          kt * 128:(kt + 1) * 128],
                                    qT_sb[:, 2 * h + dh, :],
                                    start=(dh == 0), stop=(dh == 1))
                            tmp = ap_.tile([128, 512], f32, tag="smask")
                            nc.vector.tensor_add(
                                tmp[:, :], ps_s[:, :], mask_sb[:, kt, :])
                            nc.scalar.activation(
                                expT_sb[:, kt, :], tmp[:, :], Exp)
                        po0 = psov.tile([128, 512], f32, tag="po0")
                        po1 = psov.tile([128, 512], f32, tag="po1")
                        psum = pss.tile([1, 512], f32, tag="psum")
                        for kt in range(KT):
                            nc.tensor.matmul(
                                po0[:, :], v_sb[:, h * KT + kt, 0:128],
                                expT_sb[:, kt, :],
                                start=(kt == 0), stop=(kt == KT - 1))
                            nc.tensor.matmul(
                                po1[:, :], v_sb[:, h * KT + kt, 128:256],
                                expT_sb[:, kt, :],
                                start=(kt == 0), stop=(kt == KT - 1))
                            nc.tensor.matmul(
                                psum[:, :], ones_sb[:, :], expT_sb[:, kt, :],
                                start=(kt == 0), stop=(kt == KT - 1))
                        rs = ap_.tile([1, 512], f32, tag="rs")
                        nc.vector.reciprocal(rs[:, :], psum[:, :])
                        bc = ap_.tile([128, 512], f32, tag="bc")
                        nc.gpsimd.partition_broadcast(bc[:, :], rs[:, :])
                        nc.vector.tensor_mul(
                            oT_sb[:, 2 * h + 0, :], po0[:, :], bc[:, :])
                        nc.vector.tensor_mul(
                            oT_sb[:, 2 * h + 1, :], po1[:, :], bc[:, :])

            # ---- phase B: FFN ----
            b1_sb = pp.tile([128, MT1], f32, tag="b1")
            nc.sync.dma_start(b1_sb[:, :], b1_d[:, :])
            b2_sb = pp.tile([128, KT], f32, tag="b2")
            nc.sync.dma_start(b2_sb[:, :], b2_d[:, :])

            with tc.tile_pool(name="z1", bufs=1) as zp, \
                 tc.tile_pool(name="wffn", bufs=2) as wfp, \
                 tc.tile_pool(name="fout", bufs=2) as fop:
                z1_sb = zp.tile([128, MT1, 512], f32r, tag="z1")
                for mt in range(MT1):
                    w1_sb = wfp.tile([128, D], f32r, tag="w1")
                    nc.sync.dma_start(w1_sb[:, :], w1_d[mt])
                    pz = psmm.tile([128, 512], f32, tag="mm")
                    for kt in range(KT):
                        nc.tensor.matmul(
                            pz[:, :], w1_sb[:, kt * 128:(kt + 1) * 128],
                            oT_sb[:, kt, :],
                            start=(kt == 0), stop=(kt == KT - 1))
                    nc.scalar.activation(
                        z1_sb[:, mt, :], pz[:, :], Relu,
                        bias=b1_sb[:, mt:mt + 1])
                for md in range(KT):
                    w2_sb = wfp.tile([128, FF], f32r, tag="w2")
                    nc.sync.dma_start(w2_sb[:, :], w2_d[md])
                    pf = psmm.tile([128, 512], f32, tag="mm")
                    for kt2 in range(MT1):
                        nc.tensor.matmul(
                            pf[:, :], w2_sb[:, kt2 * 128:(kt2 + 1) * 128],
                            z1_sb[:, kt2, :],
                            start=(kt2 == 0), stop=(kt2 == MT1 - 1))
                    fo = fop.tile([128, 512], f32, tag="fo")
                    nc.vector.tensor_scalar_add(fo[:, :], pf[:, :],
                                                b2_sb[:, md:md + 1])
                    nc.sync.dma_start(fT_d[md * 128:(md + 1) * 128, :],
                                      fo[:, :])
    nc.compile()
    return nc


def _build_l2():
    """LM head vocab shard: logits[4096, VC] and per-row sum(exp(logits))."""
    bass, mybir, bacc, TileContext, _ = _modules()
    f32, f32r = mybir.dt.float32, mybir.dt.float32r
    nc = bacc.Bacc("TRN2", num_devices=NCORES)

    fT_d = nc.dram_tensor("fT", [D, ROWS], f32r, kind="ExternalInput")
    wlm_d = nc.dram_tensor("wlm", [D, VCD], f32r, kind="ExternalInput")
    blm_d = nc.dram_tensor("blm", [1, VCD], f32, kind="ExternalInput")
    lg_d = nc.dram_tensor("logits", [ROWS, VCD], f32, kind="ExternalOutput")
    se_d = nc.dram_tensor("sumexp", [128, RT], f32, kind="ExternalOutput")

    Exp = mybir.ActivationFunctionType.Exp
    NCH = -(-VCD // 512)                     # 13 chunks (last ragged, even)

    with TileContext(nc) as tc:
        with tc.tile_pool(name="fT", bufs=1) as fp, \
             tc.tile_pool(name="w", bufs=2) as wp, \
             tc.tile_pool(name="work", bufs=3) as wkp, \
             tc.tile_pool(name="stats", bufs=1) as stp, \
             tc.tile_pool(name="ps", bufs=4, space="PSUM") as psp:

            fT_sb = fp.tile([128, KT, ROWS], f32r)
            nc.sync.dma_start(
                fT_sb[:, :, :], fT_d.ap().rearrange("(a p) n -> p a n", p=128))

            csum_sb = stp.tile([128, RT, NCH], f32, tag="csum")
            sums_sb = stp.tile([128, RT], f32, tag="sums")

            for n in range(NCH):
                cw = min(512, VCD - n * 512)
                wlm_sb = wp.tile([128, KT, 512], f32r, tag="wlm")
                nc.sync.dma_start(
                    wlm_sb[:, :, 0:cw],
                    wlm_d[:, n * 512:n * 512 + cw].rearrange(
                        "(a p) v -> p a v", p=128))
                bl = wkp.tile([1, 512], f32, tag="bl")
                nc.sync.dma_start(bl[:, 0:cw], blm_d[:, n * 512:n * 512 + cw])
                blb = wkp.tile([128, 512], f32, tag="blb")
                nc.gpsimd.partition_broadcast(blb[:, 0:cw], bl[:, 0:cw])
                for rt in range(RT):
                    pl = psp.tile([128, 512], f32, tag="pl")
                    for kt in range(KT):
                        nc.tensor.matmul(
                            pl[:, 0:cw], fT_sb[:, kt, rt * 128:(rt + 1) * 128],
                            wlm_sb[:, kt, 0:cw],
                            start=(kt == 0), stop=(kt == KT - 1))
                    lg = wkp.tile([128, 512], f32, tag="lg")
                    nc.vector.tensor_add(lg[:, 0:cw], pl[:, 0:cw], blb[:, 0:cw])
                    scr = wkp.tile([128, 512], f32, tag="scr")
                    nc.scalar.activation(scr[:, 0:cw], lg[:, 0:cw], Exp,
                                         accum_out=csum_sb[:, rt, n:n + 1])
                    nc.sync.dma_start(
                        lg_d[rt * 128:(rt + 1) * 128, n * 512:n * 512 + cw],
                        lg[:, 0:cw])
            for rt in range(RT):
                nc.vector.tensor_reduce(
                    sums_sb[:, rt:rt + 1], csum_sb[:, rt, :],
                    axis=mybir.AxisListType.X, op=mybir.AluOpType.add)
            nc.sync.dma_start(se_d[:, :], sums_sb[:, :])
    nc.compile()
    return nc


def _prep_l1_inputs(x, tok_emb, pos_emb, Wq, Wk, Wv, W1, b1, W2, b2):
    f4 = np.float32
    ones = np.ones((128, 1), f4)
    w1s = np.ascontiguousarray(
        W1.reshape(KT, 128, MT1, 128).transpose(2, 1, 0, 3)
        .reshape(MT1, 128, D)).astype(f4)
    b1t = np.ascontiguousarray(b1.reshape(MT1, 128).T).astype(f4)
    w2s = np.ascontiguousarray(
        W2.reshape(MT1, 128, KT, 128).transpose(2, 1, 0, 3)
        .reshape(KT, 128, FF)).astype(f4)
    b2t = np.ascontiguousarray(b2.reshape(KT, 128).T).astype(f4)
    wq = np.ascontiguousarray(Wq).astype(f4)
    wk = np.ascontiguousarray(Wk).astype(f4)
    wv = np.ascontiguousarray(Wv).astype(f4)

    # masks for the two row-halves: maskT[k, j] = 0 if k <= r0+j else NEG
    kidx = np.arange(T)[:, None]
    masks = []
    for half in range(2):
        r = (half * 512) + np.arange(512)[None, :]
        masks.append(np.where(kidx <= r, 0.0, NEG).astype(f4))

    in_maps = []
    for c in range(NCORES):
        b, half = c // 2, c % 2
        r0 = half * 512
        hb = (tok_emb[x[b]] + pos_emb).astype(f4)          # [T, D]
        in_maps.append({
            "hTq": np.ascontiguousarray(hb[r0:r0 + 512].T),
            "hTk": np.ascontiguousarray(hb.T),
            "wq": wq, "wk": wk, "wv": wv,
            "maskT": masks[half], "ones": ones,
            "w1s": w1s, "b1t": b1t, "w2s": w2s, "b2t": b2t,
        })
    return in_maps


def kernel(x, y, tok_emb, pos_emb, Wq, Wk, Wv, W1, b1, W2, b2, Wlm, blm,
           _trace=False, _timings=None):
    _, _, _, _, run_bass_kernel_spmd = _modules()
    x = np.asarray(x)
    y = np.asarray(y)
    tok_emb = np.asarray(tok_emb, np.float32)
    pos_emb = np.asarray(pos_emb, np.float32)

    if "l1" not in _CACHE:
        _CACHE["l1"] = _build_l1()
    if "l2" not in _CACHE:
        _CACHE["l2"] = _build_l2()

    # ---- launch 1: attention + FFN ----
    in1 = _prep_l1_inputs(x, tok_emb, pos_emb,
                          np.asarray(Wq, np.float32), np.asarray(Wk, np.float32),
                          np.asarray(Wv, np.float32), np.asarray(W1, np.float32),
                          np.asarray(b1, np.float32), np.asarray(W2, np.float32),
                          np.asarray(b2, np.float32))
    r1 = run_bass_kernel_spmd(_CACHE["l1"], in1, core_ids=list(range(NCORES)),
                              trace=_trace)
    if _timings is not None:
        _timings["l1_ns"] = r1.exec_time_ns

    fT_full = np.empty((D, ROWS), np.float32)
    for c in range(NCORES):
        b, half = c // 2, c % 2
        col = b * T + half * 512
        fT_full[:, col:col + 512] = r1.results[c]["fT"]

    # ---- launch 2: LM head ----
    Wlm = np.asarray(Wlm, np.float32)
    blm = np.asarray(blm, np.float32)
    wlm_pad = np.zeros((D, VC * NCORES), np.float32)
    wlm_pad[:, :V] = Wlm
    blm_pad = np.full((VC * NCORES,), 2.0 * NEG, np.float32)
    blm_pad[:V] = blm
    in2 = []
    for c in range(NCORES):
        wsh = np.zeros((D, VCD), np.float32)
        bsh = np.full((VCD,), 2.0 * NEG, np.float32)
        wsh[:, :VC] = wlm_pad[:, c * VC:(c + 1) * VC]
        bsh[:VC] = blm_pad[c * VC:(c + 1) * VC]
        in2.append({
            "fT": fT_full,
            "wlm": wsh,
            "blm": np.ascontiguousarray(bsh[None, :]),
        })
    r2 = run_bass_kernel_spmd(_CACHE["l2"], in2, core_ids=list(range(NCORES)),
                              trace=_trace)
    if _timings is not None:
        _timings["l2_ns"] = r2.exec_time_ns

    logits = np.empty((ROWS, V), np.float32)
    sumexp = np.zeros((ROWS,), np.float64)
    for c in range(NCORES):
        lo = c * VC
        hi = min(V, lo + VC)
        lg = r2.results[c]["logits"]
        logits[:, lo:hi] = lg[:, :hi - lo]
        sumexp += r2.results[c]["sumexp"].T.reshape(ROWS).astype(np.float64)

    logZ = np.log(sumexp)
    tgt = logits[np.arange(ROWS), y.reshape(ROWS).astype(np.int64)]
    loss = np.float32(np.mean(logZ - tgt.astype(np.float64)))
    return logits, loss
